# revision 8
# baseline (speedup 1.0000x reference)
"""DCT-II (norm='ortho') along axis 1 of x[8, 4096, 1024] on 8 NeuronCores.

Batch-parallel: core c computes the full DCT of batch c. Structure:
  L1..L3 Lee butterflies:  u = x[n]+x[S-1-n], v = x[n]-x[S-1-n], recursed
  on the u-side:  4096 -> 2048 -> 1024 -> 512 (dense DCT-II_512 leaf)
  Each v-side DCT-IV_N is split once via a Givens rotation stage into two
  DCT-II_{N/2}-shaped dense blocks (c-branch / s'-branch) plus an O(N)
  output combine: Y[2p] = Ct[p]+St[p], Y[2p-1] = Ct[p]-St[p], where
  St row-rearrangement, output scaling and the DST<->DCT sign flips are
  all folded into host-precomputed fp16 matrices.
  The back half of x is staged pre-reversed by the host, so the L1
  butterfly needs no on-chip reversal; deeper reversals run on the PE
  via (scaled) anti-identity matmuls. Input/output cross HBM as fp16.
"""

import sys

sys.path.insert(0, "/opt/trn_rl_repo")
import numpy as np

B, S, D = 8, 4096, 1024
W = 512          # d-chunk width
NDH = D // W     # 2 chunks
SIG = np.sqrt(2.0 / S)
SIG0 = np.sqrt(1.0 / S)

# (node size N, front tiles, kt tiles, even stride, even offset)
IV_NODES = [
    (2048, 8, 8, 4, 1),   # on v   -> odd rows
    (1024, 4, 4, 8, 2),   # on v2  -> rows 4p+2
    (512, 2, 2, 16, 4),   # on v3  -> rows 8q+4
]

_cache: dict = {}


def _T2(M):
    r = np.arange(M)[:, None]
    n = np.arange(M)[None, :]
    return np.cos(np.pi * (2 * n + 1) * r / (2 * M))


def _blocked(c):
    # [kt, 128, nt*128]: one contiguous [128 part, nt*128 free] lhsT block
    # per output tile kt (same layout as used by matmul lhsT slices).
    kt = c.shape[0] // 128
    nt = c.shape[1] // 128
    ct = c.T.astype(np.float16).reshape(nt, 128, kt, 128).transpose(2, 1, 0, 3)
    return np.ascontiguousarray(ct.reshape(kt, 128, nt * 128))


def _matrices():
    f16 = np.float16
    # dense leaf: DCT-II_512 with ortho scaling for rows y[8q]
    scale = np.full(512, SIG)
    scale[0] = SIG0
    WA = _blocked(scale[:, None] * _T2(512))

    mats = {"WA": WA}
    ads = []   # scaled anti-diagonal 128x128 matrices, 2 per front tile
    twid = []  # per-partition scalars: cosb, a2 columns per front tile
    for N, nft, _, _, _ in IV_NODES:
        M = N // 2
        n = np.arange(M)
        beta = np.pi * (2 * n + 1) / (4 * N)
        cosb, sinb = np.cos(beta), np.sin(beta)
        alt = (-1.0) ** n
        a2 = -alt * sinb          # s' = vf*a2 + vr*(alt*cosb)
        b2 = alt * cosb
        for i in range(nft):
            sl = slice(128 * i, 128 * (i + 1))
            ad_s = np.zeros((128, 128))
            adc = np.zeros((128, 128))
            m = np.arange(128)
            ad_s[127 - m, m] = sinb[sl]   # psum = vr * sinb
            adc[127 - m, m] = b2[sl]     # psum = vr * b2
            ads += [ad_s, adc]
            twid += [cosb[sl], a2[sl]]
        t2 = _T2(M)
        ws = np.zeros((M, M))
        ws[1:] = t2[M - 1: 0: -1]    # row p = T2[M-p]  (p>=1)
        ws[0] = -t2[0]               # tail-trick row: psS[0] = -sigma*T2s[0]
        mats[f"WC{M}"] = _blocked(SIG * t2)
        mats[f"WS{M}"] = _blocked(SIG * ws)
    j = np.eye(128)[::-1].copy()
    # AD tensor [nAD, 128, 128] f16 (stationary lhsT: out = lhsT.T @ rhs,
    # these are built directly as lhsT: lhsT[p, m] = weight for out row m
    # reading rhs partition p; our construction ad[127-m, m] already is lhsT)
    AD = np.ascontiguousarray(np.stack(ads).astype(f16))
    TW = np.ascontiguousarray(np.stack(twid, axis=1).astype(f16))  # [128, 28]
    return mats, AD, TW, j.astype(f16)


def _build():
    import concourse.bacc as bacc
    import concourse.mybir as mybir
    import concourse.tile as tile

    f32 = mybir.dt.float32
    f16 = mybir.dt.float16
    AL = mybir.AluOpType

    nc = bacc.Bacc("TRN2", target_bir_lowering=False, debug=False, num_devices=8)
    x_d = nc.dram_tensor("x", [NDH, S, W], f16, kind="ExternalInput").ap()
    wa_d = nc.dram_tensor("WA", [4, 128, 512], f16, kind="ExternalInput").ap()
    wc_d = {}
    ws_d = {}
    for _, _, kt, _, _ in IV_NODES:
        M = kt * 128
        wc_d[M] = nc.dram_tensor(f"WC{M}", [kt, 128, M], f16, kind="ExternalInput").ap()
        ws_d[M] = nc.dram_tensor(f"WS{M}", [kt, 128, M], f16, kind="ExternalInput").ap()
    nAD = sum(2 * nft for _, nft, _, _, _ in IV_NODES)
    ad_d = nc.dram_tensor("AD", [nAD, 128, 128], f16, kind="ExternalInput").ap()
    tw_d = nc.dram_tensor("TW", [128, nAD], f16, kind="ExternalInput").ap()
    j_d = nc.dram_tensor("J", [128, 128], f16, kind="ExternalInput").ap()
    y_d = nc.dram_tensor("y", [NDH, S, W], f16, kind="ExternalOutput").ap()

    with tile.TileContext(nc) as tc:
        with (
            tc.tile_pool(name="persist", bufs=1) as persist,
            tc.tile_pool(name="xin", bufs=1) as xin,
            tc.tile_pool(name="uv", bufs=3) as uvp,
            tc.tile_pool(name="rsb", bufs=6) as rsbp,
            tc.tile_pool(name="ssb", bufs=3) as ssbp,
            tc.tile_pool(name="sliv", bufs=4) as slivp,
            tc.tile_pool(name="ps_rev", bufs=2, space="PSUM") as ps_rev,
            tc.tile_pool(name="ps_blk", bufs=2, space="PSUM") as ps_blk,
            tc.tile_pool(name="ps_leaf", bufs=2, space="PSUM") as ps_leaf,
        ):
            # resident weights
            wa = persist.tile([128, 4, 512], f16)
            nc.scalar.dma_start(out=wa, in_=wa_d.rearrange("k p c -> p k c"))
            wc = {}
            ws = {}
            for _, _, kt, _, _ in IV_NODES:
                M = kt * 128
                wc[M] = persist.tile([128, kt, M], f16, name=f"wc{M}", tag=f"wc{M}")
                nc.scalar.dma_start(out=wc[M], in_=wc_d[M].rearrange("k p c -> p k c"))
                ws[M] = persist.tile([128, kt, M], f16, name=f"ws{M}", tag=f"ws{M}")
                nc.scalar.dma_start(out=ws[M], in_=ws_d[M].rearrange("k p c -> p k c"))
            ad = persist.tile([128, nAD, 128], f16)
            nc.scalar.dma_start(out=ad, in_=ad_d.rearrange("k p c -> p k c"))
            tw = persist.tile([128, nAD], f16)
            nc.scalar.dma_start(out=tw, in_=tw_d)
            jt = persist.tile([128, 128], f16)
            nc.scalar.dma_start(out=jt, in_=j_d)

            # persistent stage arrays (single-buffered; deps serialize dh)
            u2 = persist.tile([128, 8, W], f16, tag="u2")
            v2 = persist.tile([128, 8, W], f16, tag="v2")
            u3 = persist.tile([128, 4, W], f16, tag="u3")
            v3 = persist.tile([128, 4, W], f16, tag="v3")
            cs = {2048: (persist.tile([128, 8, W], f16, name="c1", tag="c1"),
                         persist.tile([128, 8, W], f16, name="s1", tag="s1")),
                  1024: (persist.tile([128, 4, W], f16, name="c2", tag="c2"),
                         persist.tile([128, 4, W], f16, name="s2", tag="s2")),
                  512: (persist.tile([128, 2, W], f16, name="c3", tag="c3"),
                        persist.tile([128, 2, W], f16, name="s3", tag="s3"))}
            psb = {2048: (persist.tile([128, 8, W], f16, name="P1", tag="P1"),
                          persist.tile([128, 8, W], f16, name="D1", tag="D1")),
                   1024: (persist.tile([128, 4, W], f16, name="P2", tag="P2"),
                          persist.tile([128, 4, W], f16, name="D2", tag="D2")),
                   512: (persist.tile([128, 2, W], f16, name="P3", tag="P3"),
                         persist.tile([128, 2, W], f16, name="D3", tag="D3"))}
            ysb = persist.tile([128, 4, W], f16, tag="ysb")

            # AD matrix index offsets per node
            ad_off = {}
            off = 0
            for N, nft, _, _, _ in IV_NODES:
                ad_off[N] = off
                off += 2 * nft

            for dh in range(NDH):
                xv = x_d[dh]  # [4096, W]
                # ---- x loads: 4 blocks of 8 row-tiles (1 MiB each) ----
                xf = [xin.tile([128, 8, W], f16, name=f"xf{b}", tag=f"xf{b}") for b in range(2)]
                xb = [xin.tile([128, 8, W], f16, name=f"xb{b}", tag=f"xb{b}") for b in range(2)]
                for b in range(2):
                    nc.sync.dma_start(
                        out=xf[b],
                        in_=xv[1024 * b: 1024 * (b + 1)].rearrange(
                            "(t p) f -> p t f", p=128))
                    nc.sync.dma_start(
                        out=xb[b],
                        in_=xv[2048 + 1024 * b: 2048 + 1024 * (b + 1)].rearrange(
                            "(t p) f -> p t f", p=128))

                # ---- fused L1 + L2 + rot2048 over pairs (i, 15-i) ----
                c1_t, s1_t = cs[2048]
                ao0 = ad_off[2048]
                for i in range(8):
                    ulo = uvp.tile([128, W], f16, tag="ulo")
                    uhi = uvp.tile([128, W], f16, tag="uhi")
                    vlo = uvp.tile([128, W], f16, tag="vlo")
                    vhi = uvp.tile([128, W], f16, tag="vhi")
                    nc.gpsimd.tensor_add(ulo, xf[0][:, i, :], xb[0][:, i, :])
                    nc.gpsimd.tensor_sub(vlo, xf[0][:, i, :], xb[0][:, i, :])
                    nc.gpsimd.tensor_add(uhi, xf[1][:, 7 - i, :], xb[1][:, 7 - i, :])
                    nc.gpsimd.tensor_sub(vhi, xf[1][:, 7 - i, :], xb[1][:, 7 - i, :])
                    # L2 for q=i: u2[i] = ulo + rev(uhi), v2[i] = ulo - rev(uhi)
                    ps = ps_rev.tile([128, W], f32, tag="rev")
                    nc.tensor.matmul(ps, jt, uhi, start=True, stop=True)
                    r = rsbp.tile([128, W], f16, tag="r")
                    nc.scalar.copy(out=r, in_=ps)
                    nc.gpsimd.tensor_add(u2[:, i, :], ulo, r)
                    nc.gpsimd.tensor_sub(v2[:, i, :], ulo, r)
                    # rot2048 tile i: c1[i], s1[i]
                    ao = ao0 + 2 * i
                    ps1 = ps_rev.tile([128, W], f32, tag="rev")
                    nc.tensor.matmul(ps1, ad[:, ao, :], vhi, start=True, stop=True)
                    r1 = rsbp.tile([128, W], f16, tag="r")
                    nc.scalar.copy(out=r1, in_=ps1)
                    nc.vector.scalar_tensor_tensor(
                        out=c1_t[:, i, :], in0=vlo, scalar=tw[:, ao: ao + 1],
                        in1=r1, op0=AL.mult, op1=AL.add)
                    ps2 = ps_rev.tile([128, W], f32, tag="rev")
                    nc.tensor.matmul(ps2, ad[:, ao + 1, :], vhi, start=True, stop=True)
                    r2 = rsbp.tile([128, W], f16, tag="r")
                    nc.scalar.copy(out=r2, in_=ps2)
                    nc.vector.scalar_tensor_tensor(
                        out=s1_t[:, i, :], in0=vlo, scalar=tw[:, ao + 1: ao + 2],
                        in1=r2, op0=AL.mult, op1=AL.add)

                def rot(N, src):
                    """Rotation stage: src [128, nft*2, W] -> c, s' arrays."""
                    nft = [n for n in IV_NODES if n[0] == N][0][1]
                    c_t, s_t = cs[N]
                    for i in range(nft):
                        ao = ad_off[N] + 2 * i
                        ps1 = ps_rev.tile([128, W], f32, tag="rev")
                        nc.tensor.matmul(ps1, ad[:, ao, :], src[:, 2 * nft - 1 - i, :],
                                         start=True, stop=True)
                        r1 = rsbp.tile([128, W], f16, tag="r")
                        nc.scalar.copy(out=r1, in_=ps1)
                        nc.vector.scalar_tensor_tensor(
                            out=c_t[:, i, :], in0=src[:, i, :], scalar=tw[:, ao: ao + 1],
                            in1=r1, op0=AL.mult, op1=AL.add)
                        ps2 = ps_rev.tile([128, W], f32, tag="rev")
                        nc.tensor.matmul(ps2, ad[:, ao + 1, :], src[:, 2 * nft - 1 - i, :],
                                         start=True, stop=True)
                        r2 = rsbp.tile([128, W], f16, tag="r")
                        nc.scalar.copy(out=r2, in_=ps2)
                        nc.vector.scalar_tensor_tensor(
                            out=s_t[:, i, :], in0=src[:, i, :], scalar=tw[:, ao + 1: ao + 2],
                            in1=r2, op0=AL.mult, op1=AL.add)

                def lee(src, nsrc, dst_u, dst_v):
                    """One Lee butterfly level on src [128, nsrc, W]."""
                    for q in range(nsrc // 2):
                        ps = ps_rev.tile([128, W], f32, tag="rev")
                        nc.tensor.matmul(ps, jt, src[:, nsrc - 1 - q, :],
                                         start=True, stop=True)
                        r = rsbp.tile([128, W], f16, tag="r")
                        nc.scalar.copy(out=r, in_=ps)
                        nc.gpsimd.tensor_add(dst_u[:, q, :], src[:, q, :], r)
                        nc.gpsimd.tensor_sub(dst_v[:, q, :], src[:, q, :], r)

                def iv_blocks(N):
                    nft, kt, estride, eoff = [
                        (n[1], n[2], n[3], n[4]) for n in IV_NODES if n[0] == N][0]
                    M = kt * 128
                    c_t, s_t = cs[N]
                    P_t, D_t = psb[N]
                    for k in range(kt):
                        psC = ps_blk.tile([128, W], f32, tag="psC")
                        for nt2 in range(kt):
                            nc.tensor.matmul(
                                psC, wc[M][:, k, nt2 * 128:(nt2 + 1) * 128],
                                c_t[:, nt2, :], start=(nt2 == 0), stop=(nt2 == kt - 1))
                        psS = ps_blk.tile([128, W], f32, tag="psS")
                        for nt2 in range(kt):
                            nc.tensor.matmul(
                                psS, ws[M][:, k, nt2 * 128:(nt2 + 1) * 128],
                                s_t[:, nt2, :], start=(nt2 == 0), stop=(nt2 == kt - 1))
                        sb = ssbp.tile([128, W], f32, tag="sb")
                        nc.scalar.copy(out=sb, in_=psS)
                        nc.vector.tensor_add(P_t[:, k, :], psC, sb)
                        nc.vector.tensor_sub(D_t[:, k, :], psC, sb)
                        if k == 0:
                            # P[0] = psC[0]; tail row y[e*(2M-1)/2...] = psS[0]
                            nc.scalar.copy(out=P_t[0:1, 0, :], in_=psC[0:1, :])
                            tl = slivp.tile([1, W], f16, tag="tl")
                            nc.scalar.copy(out=tl, in_=psS[0:1, :])
                            tview = y_d[dh].rearrange("(r e) f -> e r f", e=estride)
                            nc.sync.dma_start(
                                out=tview[eoff + estride // 2, S // estride - 1: S // estride, :],
                                in_=tl)
                    # stores
                    pview = y_d[dh].rearrange("(r e) f -> e r f", e=estride)[eoff]
                    pview = pview.rearrange("(n p) f -> p n f", p=128)
                    nc.sync.dma_start(out=pview[:, 0:kt, :], in_=P_t[:, 0:kt, :])
                    doff = eoff + estride // 2  # D[p] -> row e*(p-1)+doff = e*p - e/2 + eoff
                    dview = y_d[dh].rearrange("(r e) f -> e r f", e=estride)[doff]
                    dview = dview.rearrange("(n p) f -> p n f", p=128)
                    nc.sync.dma_start(out=dview[0:127, 0:kt, :], in_=D_t[1:128, 0:kt, :])
                    if kt > 1:
                        nc.sync.dma_start(out=dview[127:128, 0:kt - 1, :],
                                          in_=D_t[0:1, 1:kt, :])

                # ---- remaining butterflies & blocks ----
                iv_blocks(2048)
                rot(1024, v2)
                lee(u2, 8, u3, v3)
                rot(512, v3)
                iv_blocks(1024)
                iv_blocks(512)
                # ---- dense leaf II512 -> y[8q] ----
                for k in range(4):
                    ps = ps_leaf.tile([128, W], f32, tag="leaf")
                    for nt2 in range(4):
                        nc.tensor.matmul(ps, wa[:, k, nt2 * 128:(nt2 + 1) * 128],
                                         u3[:, nt2, :], start=(nt2 == 0), stop=(nt2 == 3))
                    nc.scalar.copy(out=ysb[:, k, :], in_=ps)
                lview = y_d[dh].rearrange("(r e) f -> e r f", e=8)[0]
                lview = lview.rearrange("(n p) f -> p n f", p=128)
                nc.sync.dma_start(out=lview[:, 0:4, :], in_=ysb[:, 0:4, :])
    nc.compile()
    return nc


def _get_nc():
    if "nc" not in _cache:
        _cache["nc"] = _build()
        _cache["mats"] = _matrices()
    return _cache["nc"]


def _run(x: np.ndarray, trace: bool = False):
    from concourse.bass_utils import run_bass_kernel_spmd

    nc = _get_nc()
    mats, AD, TW, J = _cache["mats"]
    x = np.asarray(x)
    in_maps = []
    for c in range(B):
        xc = x[c].astype(np.float16)
        xs = np.concatenate([xc[: S // 2], xc[S // 2:][::-1]], axis=0)
        xs = np.ascontiguousarray(xs.reshape(S, NDH, W).transpose(1, 0, 2))
        m = {"x": xs, "WA": mats["WA"], "AD": AD, "TW": TW, "J": J}
        for _, _, kt, _, _ in IV_NODES:
            M = kt * 128
            m[f"WC{M}"] = mats[f"WC{M}"]
            m[f"WS{M}"] = mats[f"WS{M}"]
        in_maps.append(m)
    res = run_bass_kernel_spmd(
        nc, in_maps, list(range(B)), trace=trace, trace_cores=[0] if trace else None
    )
    out = np.empty((B, S, D), dtype=np.float32)
    for c in range(B):
        yc = res.results[c]["y"]  # [NDH, S, W] f16
        out[c] = yc.transpose(1, 0, 2).reshape(S, D).astype(np.float32)
    return out, res


def kernel(x: np.ndarray) -> np.ndarray:
    out, _ = _run(x, trace=False)
    return out


# revision 10
# speedup vs baseline: 1.6718x; 1.6718x over previous
"""DCT-II (norm='ortho') along axis 1 of x[8, 4096, 1024] on 8 NeuronCores.

Batch-parallel: core c computes the full DCT of batch c. Structure:
  Lee butterflies on the DCT-II side (4096 -> 2048 -> 1024 -> 512 leaf),
  and each DCT-IV_N split once via a Givens rotation stage into two
  DCT-II_{N/2}-shaped dense fp16 matmul blocks plus an O(N) combine
  (Y[2p] = Ct[p]+St[p], Y[2p-1] = Ct[p]-St[p]).  All row scalings, the
  DST<->DCT flips and the St row-rearrangement are folded into
  host-precomputed matrices.  The back half of x is staged pre-reversed;
  deeper reversals and the rotation twiddles run on the PE as (scaled)
  anti-diagonal/diagonal matmul pairs accumulated in PSUM.  x and y cross
  HBM as fp16 in partition-major layout (every DMA is contiguous per
  partition); the host performs the final output row interleave.
"""

import sys

sys.path.insert(0, "/opt/trn_rl_repo")
import numpy as np

B, S, D = 8, 4096, 1024
SIG = np.sqrt(2.0 / S)
SIG0 = np.sqrt(1.0 / S)

# (node size N, n front tiles, kt output tiles)
IV_NODES = [(2048, 8, 8), (1024, 4, 4), (512, 2, 2)]

# y slot map (32 slots of [128, 1024] in HBM, partition-major)
SLOT = {"P1": 0, "D1": 8, "P2": 16, "D2": 20, "P3": 24, "D3": 26, "LF": 28}

_cache: dict = {}


def _T2(M):
    r = np.arange(M)[:, None]
    n = np.arange(M)[None, :]
    return np.cos(np.pi * (2 * n + 1) * r / (2 * M))


def _blocked(c):
    # [kt, 128, nt*128] lhsT blocks: blk[k][i][nt*128+j] = c[128k+j, 128nt+i]
    kt = c.shape[0] // 128
    nt = c.shape[1] // 128
    ct = c.T.astype(np.float16).reshape(nt, 128, kt, 128).transpose(2, 1, 0, 3)
    return np.ascontiguousarray(ct.reshape(kt, 128, nt * 128))


def _pmajor(blk):
    # [kt, 128, C] -> [128, kt*C] (contiguous per partition)
    kt, _, C = blk.shape
    return np.ascontiguousarray(blk.transpose(1, 0, 2).reshape(128, kt * C))


def _matrices():
    f16 = np.float16
    scale = np.full(512, SIG)
    scale[0] = SIG0
    res = {}
    res["WA"] = _pmajor(_blocked(scale[:, None] * _T2(512)))
    rot = []   # per front tile: Dcos, ADsin, Da2, ADb2  (lhsT 128x128 each)
    for N, nft, kt in IV_NODES:
        M = N // 2
        n = np.arange(M)
        beta = np.pi * (2 * n + 1) / (4 * N)
        cosb, sinb = np.cos(beta), np.sin(beta)
        alt = (-1.0) ** n
        a2 = -alt * sinb
        b2 = alt * cosb
        for i in range(nft):
            sl = slice(128 * i, 128 * (i + 1))
            m = np.arange(128)
            dc = np.zeros((128, 128)); dc[m, m] = cosb[sl]
            asn = np.zeros((128, 128)); asn[127 - m, m] = sinb[sl]
            da = np.zeros((128, 128)); da[m, m] = a2[sl]
            ab = np.zeros((128, 128)); ab[127 - m, m] = b2[sl]
            rot += [dc, asn, da, ab]
        t2 = _T2(M)
        ws = np.zeros((M, M))
        ws[1:] = t2[M - 1: 0: -1]
        ws[0] = -t2[0]
        if N == 2048:
            res[f"WC{M}"] = _blocked(SIG * t2)           # [kt, 128, 1024]
            res[f"WS{M}"] = _blocked(SIG * ws)
        else:
            res[f"WC{M}"] = _pmajor(_blocked(SIG * t2))  # [128, kt*CW]
            res[f"WS{M}"] = _pmajor(_blocked(SIG * ws))
    j = np.eye(128)[::-1]
    res["ROT"] = _pmajor(np.stack(rot).astype(f16))   # [128, nrot*128]
    res["J"] = np.ascontiguousarray(j.astype(f16))
    return res


def _build():
    import concourse.bacc as bacc
    import concourse.mybir as mybir
    import concourse.tile as tile

    f32 = mybir.dt.float32
    f16 = mybir.dt.float16

    nrot = sum(4 * nft for _, nft, _ in IV_NODES)
    nc = bacc.Bacc("TRN2", target_bir_lowering=False, debug=False, num_devices=8)
    x_d = nc.dram_tensor("x", [8, 128, 4 * D], f16, kind="ExternalInput").ap()
    wa_d = nc.dram_tensor("WA", [128, 4 * 512], f16, kind="ExternalInput").ap()
    wc1_d = nc.dram_tensor("WC1024", [8, 128, 1024], f16, kind="ExternalInput").ap()
    ws1_d = nc.dram_tensor("WS1024", [8, 128, 1024], f16, kind="ExternalInput").ap()
    wc2_d = nc.dram_tensor("WC512", [128, 4 * 512], f16, kind="ExternalInput").ap()
    ws2_d = nc.dram_tensor("WS512", [128, 4 * 512], f16, kind="ExternalInput").ap()
    wc3_d = nc.dram_tensor("WC256", [128, 2 * 256], f16, kind="ExternalInput").ap()
    ws3_d = nc.dram_tensor("WS256", [128, 2 * 256], f16, kind="ExternalInput").ap()
    rot_d = nc.dram_tensor("ROT", [128, nrot * 128], f16, kind="ExternalInput").ap()
    j_d = nc.dram_tensor("J", [128, 128], f16, kind="ExternalInput").ap()
    y_d = nc.dram_tensor("y", [128, 32, D], f16, kind="ExternalOutput").ap()

    rot_off = {}
    off = 0
    for N, nft, _ in IV_NODES:
        rot_off[N] = off
        off += 4 * nft

    with tile.TileContext(nc) as tc:
        with (
            tc.tile_pool(name="persist", bufs=1) as persist,
            tc.tile_pool(name="xin", bufs=1) as xin,
            tc.tile_pool(name="uv", bufs=2) as uvp,
            tc.tile_pool(name="rsb", bufs=4) as rsbp,
            tc.tile_pool(name="ssb", bufs=3) as ssbp,
            tc.tile_pool(name="cw", bufs=2) as cwp,
            tc.tile_pool(name="pd", bufs=3) as pdp,
            tc.tile_pool(name="ps_rev", bufs=3, space="PSUM") as ps_rev,
            tc.tile_pool(name="ps_c", bufs=2, space="PSUM") as ps_c,
            tc.tile_pool(name="ps_s", bufs=2, space="PSUM") as ps_s,
            tc.tile_pool(name="ps_leaf", bufs=1, space="PSUM") as ps_leaf,
        ):
            rt = persist.tile([128, nrot, 128], f16)
            nc.sync.dma_start(out=rt, in_=rot_d.rearrange("p (k c) -> p k c", k=nrot))
            jt = persist.tile([128, 128], f16)
            nc.sync.dma_start(out=jt, in_=j_d)

            u2 = persist.tile([128, 8, D], f16, tag="u2")
            v2 = persist.tile([128, 8, D], f16, tag="v2")
            u3 = persist.tile([128, 4, D], f16, tag="u3")
            v3 = persist.tile([128, 4, D], f16, tag="v3")
            cs = {2048: (persist.tile([128, 8, D], f16, name="c1", tag="c1"),
                         persist.tile([128, 8, D], f16, name="s1", tag="s1")),
                  1024: (persist.tile([128, 4, D], f16, name="c2", tag="c2"),
                         persist.tile([128, 4, D], f16, name="s2", tag="s2")),
                  512: (persist.tile([128, 2, D], f16, name="c3", tag="c3"),
                        persist.tile([128, 2, D], f16, name="s3", tag="s3"))}

            # ---- x loads: 8 blocks of 4 row-tiles (1 MiB, contiguous/partition)
            # front tiles 0..15 = blocks 0..3, back(rev) tiles = blocks 4..7
            xt = {}
            for b, tag in ((0, "fA"), (4, "bA"), (3, "fB"), (7, "bB"),
                           (1, "fA"), (5, "bA"), (2, "fB"), (6, "bB")):
                t = xin.tile([128, 4, D], f16, name=f"x{b}", tag=tag)
                nc.sync.dma_start(out=t, in_=x_d[b].rearrange("p (t f) -> p t f", t=4))
                xt[b] = t

            def rot_pair(N, i, vlo, vhi):
                """c[i], s[i] via 2-matmul PSUM accumulation per 512 chunk."""
                ro = rot_off[N] + 4 * i
                c_t, s_t = cs[N]
                for ch in range(2):
                    sl = slice(ch * 512, (ch + 1) * 512)
                    cps = ps_rev.tile([128, 512], f32, tag="rev")
                    nc.tensor.matmul(cps, rt[:, ro, :], vlo[:, sl], start=True, stop=False)
                    nc.tensor.matmul(cps, rt[:, ro + 1, :], vhi[:, sl], start=False, stop=True)
                    nc.scalar.copy(out=c_t[:, i, sl], in_=cps)
                    sps = ps_rev.tile([128, 512], f32, tag="rev")
                    nc.tensor.matmul(sps, rt[:, ro + 2, :], vlo[:, sl], start=True, stop=False)
                    nc.tensor.matmul(sps, rt[:, ro + 3, :], vhi[:, sl], start=False, stop=True)
                    nc.scalar.copy(out=s_t[:, i, sl], in_=sps)

            # ---- fused L1 + L2 + rot2048 over pairs (i, 15-i) ----
            for i in range(8):
                fl, fh = xt[i // 4], xt[3 - i // 4]
                bl, bh = xt[4 + i // 4], xt[7 - i // 4]
                slo, shi = i % 4, (3 - i) % 4
                ulo = uvp.tile([128, D], f16, tag="ulo")
                uhi = uvp.tile([128, D], f16, tag="uhi")
                vlo = uvp.tile([128, D], f16, tag="vlo")
                vhi = uvp.tile([128, D], f16, tag="vhi")
                nc.vector.tensor_add(ulo, fl[:, slo, :], bl[:, slo, :])
                nc.vector.tensor_sub(vlo, fl[:, slo, :], bl[:, slo, :])
                nc.vector.tensor_add(uhi, fh[:, shi, :], bh[:, shi, :])
                nc.vector.tensor_sub(vhi, fh[:, shi, :], bh[:, shi, :])
                # L2 (q=i): u2[i] = ulo + rev(uhi), v2[i] = ulo - rev(uhi)
                r = rsbp.tile([128, D], f16, tag="r")
                for ch in range(2):
                    sl = slice(ch * 512, (ch + 1) * 512)
                    ps = ps_rev.tile([128, 512], f32, tag="rev")
                    nc.tensor.matmul(ps, jt, uhi[:, sl], start=True, stop=True)
                    nc.scalar.copy(out=r[:, sl], in_=ps)
                nc.vector.tensor_add(u2[:, i, :], ulo, r)
                nc.vector.tensor_sub(v2[:, i, :], ulo, r)
                rot_pair(2048, i, vlo, vhi)

            # ---- fused L3 + rot1024 over u2/v2 pairs (o, 7-o) ----
            for o in range(4):
                r = rsbp.tile([128, D], f16, tag="r")
                for ch in range(2):
                    sl = slice(ch * 512, (ch + 1) * 512)
                    ps = ps_rev.tile([128, 512], f32, tag="rev")
                    nc.tensor.matmul(ps, jt, u2[:, 7 - o, sl], start=True, stop=True)
                    nc.scalar.copy(out=r[:, sl], in_=ps)
                nc.vector.tensor_add(u3[:, o, :], u2[:, o, :], r)
                nc.vector.tensor_sub(v3[:, o, :], u2[:, o, :], r)
                rot_pair(1024, o, v2[:, o, :], v2[:, 7 - o, :])
            for i in range(2):
                rot_pair(512, i, v3[:, i, :], v3[:, 3 - i, :])

            def iv_blocks(N, wc_src, ws_src, pslot, dslot):
                kt = [n[2] for n in IV_NODES if n[0] == N][0]
                c_t, s_t = cs[N]
                CW = kt * 128  # lhsT block width
                for k in range(kt):
                    wck = cwp.tile([128, 1024], f16, tag="wck")
                    wsk = cwp.tile([128, 1024], f16, tag="wsk")
                    if N == 2048:
                        nc.sync.dma_start(out=wck, in_=wc_src[k])
                        nc.sync.dma_start(out=wsk, in_=ws_src[k])
                    else:
                        nc.sync.dma_start(out=wck[:, 0:CW],
                                          in_=wc_src[:, k * CW:(k + 1) * CW])
                        nc.sync.dma_start(out=wsk[:, 0:CW],
                                          in_=ws_src[:, k * CW:(k + 1) * CW])
                    P = pdp.tile([128, D], f16, tag="P")
                    Dm = pdp.tile([128, D], f16, tag="Dm")
                    for ch in range(2):
                        sl = slice(ch * 512, (ch + 1) * 512)
                        psC = ps_c.tile([128, 512], f32, tag="psC")
                        for nt2 in range(kt):
                            nc.tensor.matmul(
                                psC, wck[:, nt2 * 128:(nt2 + 1) * 128],
                                c_t[:, nt2, sl], start=(nt2 == 0), stop=(nt2 == kt - 1))
                        psS = ps_s.tile([128, 512], f32, tag="psS")
                        for nt2 in range(kt):
                            nc.tensor.matmul(
                                psS, wsk[:, nt2 * 128:(nt2 + 1) * 128],
                                s_t[:, nt2, sl], start=(nt2 == 0), stop=(nt2 == kt - 1))
                        sb = ssbp.tile([128, 512], f32, tag="sb")
                        nc.scalar.copy(out=sb, in_=psS)
                        nc.vector.tensor_add(P[:, sl], psC, sb)
                        nc.vector.tensor_sub(Dm[:, sl], psC, sb)
                    nc.sync.dma_start(out=y_d[:, pslot + k, :], in_=P)
                    nc.sync.dma_start(out=y_d[:, dslot + k, :], in_=Dm)

            iv_blocks(2048, wc1_d, ws1_d, SLOT["P1"], SLOT["D1"])
            iv_blocks(1024, wc2_d, ws2_d, SLOT["P2"], SLOT["D2"])
            iv_blocks(512, wc3_d, ws3_d, SLOT["P3"], SLOT["D3"])

            # ---- dense leaf II512 -> slots LF..LF+3 ----
            for k in range(4):
                wak = cwp.tile([128, 1024], f16, tag="wck")
                nc.sync.dma_start(out=wak[:, 0:512], in_=wa_d[:, k * 512:(k + 1) * 512])
                ysk = pdp.tile([128, D], f16, tag="P")
                for ch in range(2):
                    sl = slice(ch * 512, (ch + 1) * 512)
                    ps = ps_leaf.tile([128, 512], f32, tag="leaf")
                    for nt2 in range(4):
                        nc.tensor.matmul(ps, wak[:, nt2 * 128:(nt2 + 1) * 128],
                                         u3[:, nt2, sl], start=(nt2 == 0), stop=(nt2 == 3))
                    nc.scalar.copy(out=ysk[:, sl], in_=ps)
                nc.sync.dma_start(out=y_d[:, SLOT["LF"] + k, :], in_=ysk)
    nc.compile()
    return nc


def _get_nc():
    if "nc" not in _cache:
        _cache["nc"] = _build()
        _cache["mats"] = _matrices()
    return _cache["nc"]


def _reassemble(yraw):
    """yraw [128, 32, 1024] f16 -> y [4096, 1024] f32."""
    y = np.empty((S, D), dtype=np.float32)
    specs = [(2048, 8, "P1", "D1", 2, 1), (1024, 4, "P2", "D2", 4, 2),
             (512, 2, "P3", "D3", 8, 4)]
    for N, kt, pk, dk, lstride, loff in specs:
        M = N // 2
        P = yraw[:, SLOT[pk]: SLOT[pk] + kt, :].astype(np.float32)
        Dm = yraw[:, SLOT[dk]: SLOT[dk] + kt, :].astype(np.float32)
        P = P.transpose(1, 0, 2).reshape(M, D)
        Dm = Dm.transpose(1, 0, 2).reshape(M, D)
        p = np.arange(M)
        # local node rows: P[p] -> 2p, D[p] -> 2p-1 (p>=1), tail at 2M-1
        # global: local i -> lstride*i + loff
        y[lstride * 2 * p + loff] = P
        y[lstride * (2 * p[1:] - 1) + loff] = Dm[1:]
        y[loff] = 0.5 * (P[0] + Dm[0])                    # psC[0]
        y[lstride * (2 * M - 1) + loff] = 0.5 * (P[0] - Dm[0])  # psS[0] = tail
    LF = yraw[:, SLOT["LF"]: SLOT["LF"] + 4, :].astype(np.float32)
    y[0::8] = LF.transpose(1, 0, 2).reshape(512, D)
    return y


def _run(x: np.ndarray, trace: bool = False):
    from concourse.bass_utils import run_bass_kernel_spmd

    nc = _get_nc()
    mats = _cache["mats"]
    x = np.asarray(x)
    in_maps = []
    for c in range(B):
        xc = x[c].astype(np.float16)
        xs = np.concatenate([xc[: S // 2], xc[S // 2:][::-1]], axis=0)
        xpm = np.ascontiguousarray(
            xs.reshape(8, 4, 128, D).transpose(0, 2, 1, 3).reshape(8, 128, 4 * D))
        m = {"x": xpm, "WA": mats["WA"], "WC1024": mats["WC1024"],
             "WS1024": mats["WS1024"], "WC512": mats["WC512"],
             "WS512": mats["WS512"], "WC256": mats["WC256"],
             "WS256": mats["WS256"], "ROT": mats["ROT"], "J": mats["J"]}
        in_maps.append(m)
    res = run_bass_kernel_spmd(
        nc, in_maps, list(range(B)), trace=trace, trace_cores=[0] if trace else None
    )
    out = np.empty((B, S, D), dtype=np.float32)
    for c in range(B):
        out[c] = _reassemble(res.results[c]["y"])
    return out, res


def kernel(x: np.ndarray) -> np.ndarray:
    out, _ = _run(x, trace=False)
    return out
